# revision 27
# baseline (speedup 1.0000x reference)
"""Trainium2 Bass kernel for a 4-direction cross selective scan (VMamba SS2D).

Strategy: 8 NeuronCores, one (batch, direction) pair per core — B=2 x 4
directions. Each core runs an identical S6 selective-scan program over its
own pre-permuted (L, C) sequence; the host un-permutes and averages the four
directional outputs per batch.

Device layout: channels on partitions (two halves of 128), time on the free
dimension in chunks of T. The recurrence h_t = dA_t*h + dBx_t runs on the
native DVE tensor_tensor_scan (fp32 internal state), dA = exp(A*delta) on the
ACT engine with per-partition scale, softplus via exp+ln (one ACT table), and
the n-contraction y = sum_n C_n*h_n as a strided tensor_reduce.
"""

import sys

try:
    import concourse.bass as bass  # noqa: F401
except ImportError:
    sys.path.insert(0, "/opt/trn_rl_repo")

import numpy as np
import ml_dtypes
import concourse.mybir as mybir
import concourse.bacc as bacc
from concourse import tile
from concourse.bass_utils import run_bass_kernel_spmd

import os

B_, C_, H_, W_, N_ = 2, 256, 64, 64, 16
L_ = H_ * W_
T_ = int(os.environ.get("SS_T", 512))    # time-chunk width (free dim)
NCHUNK = L_ // T_
NCORES = 8
F32 = mybir.dt.float32
BF16 = mybir.dt.bfloat16
DT_LIN = BF16             # dtype of the linear factors (u, B, C, dBx, h, y')
AF = mybir.ActivationFunctionType
OP = mybir.AluOpType

_prog_cache = {}

NCHUNK_BUILD = int(os.environ.get("SS_NCHUNK", NCHUNK))
BC_VIA_DMA = os.environ.get("SS_BC", "dma") == "dma"
INPLACE_LN = os.environ.get("SS_LN", "sep") == "inplace"
REDUCE_ENG = os.environ.get("SS_RED", "pool")


def _build_program():
    if "nc" in _prog_cache:
        return _prog_cache["nc"]
    nc = bacc.Bacc("TRN2", target_bir_lowering=False, debug=False,
                   num_devices=NCORES)
    xt_d = nc.dram_tensor("xt", [C_, L_], F32, kind="ExternalInput")
    wd_d = nc.dram_tensor("wd", [C_, C_], F32, kind="ExternalInput")
    brow_d = nc.dram_tensor("brow", [N_, L_], DT_LIN, kind="ExternalInput")
    crow_d = nc.dram_tensor("crow", [N_, L_], DT_LIN, kind="ExternalInput")
    asc_d = nc.dram_tensor("asc", [128, 2 * N_], F32, kind="ExternalInput")
    bsc_d = nc.dram_tensor("bsc", [128, 2], F32, kind="ExternalInput")
    yt_d = nc.dram_tensor("yt", [C_, L_], F32, kind="ExternalOutput")

    with tile.TileContext(nc) as tc:
        with (
            tc.tile_pool(name="const", bufs=1) as cpool,
            tc.tile_pool(name="work", bufs=2) as wpool,
            tc.tile_pool(name="big", bufs=2) as bigpool,
            tc.tile_pool(name="big1", bufs=1) as big1pool,
            tc.tile_pool(name="da", bufs=4) as dapool,
            tc.tile_pool(name="one", bufs=1) as onepool,
            tc.tile_pool(name="psum", bufs=3, space="PSUM") as pspool,
        ):
            wd0 = cpool.tile([128, C_], F32, tag="wd0")
            nc.sync.dma_start(wd0[:], wd_d[0:128, :])
            wd1 = cpool.tile([128, C_], F32, tag="wd1")
            nc.sync.dma_start(wd1[:], wd_d[128:256, :])
            asc = cpool.tile([128, 2 * N_], F32, tag="asc")
            nc.sync.dma_start(asc[:], asc_d[:])
            bsc = cpool.tile([128, 2], F32, tag="bsc")
            nc.sync.dma_start(bsc[:], bsc_d[:])
            state = cpool.tile([128, 2 * N_], F32, tag="state")

            for k in range(NCHUNK_BUILD):
                sl = slice(k * T_, (k + 1) * T_)
                xts = []
                for h in range(2):
                    xt_h = wpool.tile([128, T_], F32, tag=f"xt{h}")
                    nc.sync.dma_start(xt_h[:], xt_d[h * 128:(h + 1) * 128, sl])
                    xts.append(xt_h)
                # Broadcast B/C time-rows to all 128 partitions straight from
                # DRAM (step-0 partition AP on the DMA source).
                bbc = bigpool.tile([128, N_, T_], DT_LIN, tag="bbc")
                cbc = bigpool.tile([128, N_, T_], DT_LIN, tag="cbc")
                if BC_VIA_DMA:
                    nc.sync.dma_start(
                        bbc[:], brow_d[:, sl].unsqueeze(0).broadcast_to([128, N_, T_])
                    )
                    nc.sync.dma_start(
                        cbc[:], crow_d[:, sl].unsqueeze(0).broadcast_to([128, N_, T_])
                    )
                else:
                    brr = wpool.tile([1, N_, T_], DT_LIN, tag="brr")
                    nc.sync.dma_start(brr[0:1, :, :], brow_d[:, sl])
                    nc.gpsimd.partition_broadcast(
                        bbc[:].rearrange("p n t -> p (n t)"),
                        brr[0:1].rearrange("p n t -> p (n t)"))
                    crr = wpool.tile([1, N_, T_], DT_LIN, tag="crr")
                    nc.sync.dma_start(crr[0:1, :, :], crow_d[:, sl])
                    nc.gpsimd.partition_broadcast(
                        cbc[:].rearrange("p n t -> p (n t)"),
                        crr[0:1].rearrange("p n t -> p (n t)"))
                for h in range(2):
                    psd = pspool.tile([128, T_], F32, tag="psd")
                    nc.tensor.matmul(psd[:], wd0[:, h * 128:(h + 1) * 128],
                                     xts[0][:], start=True, stop=False)
                    nc.tensor.matmul(psd[:], wd1[:, h * 128:(h + 1) * 128],
                                     xts[1][:], start=False, stop=True)
                    # softplus(z + b) = ln(1 + exp(z + b)); Exp and Ln live in
                    # the same ACT table so no table reloads.
                    delta = wpool.tile([128, T_], F32, tag=f"delta{h}")
                    if INPLACE_LN:
                        nc.scalar.activation(delta[:], psd[:], AF.Exp,
                                             bias=bsc[:, h:h + 1])
                        nc.scalar.activation(delta[:], delta[:], AF.Ln, bias=1.0)
                    else:
                        esb = onepool.tile([128, T_], F32, tag="esb")
                        nc.scalar.activation(esb[:], psd[:], AF.Exp,
                                             bias=bsc[:, h:h + 1])
                        nc.scalar.activation(delta[:], esb[:], AF.Ln, bias=1.0)
                    u_h = wpool.tile([128, T_], DT_LIN, tag=f"u{h}")
                    nc.gpsimd.tensor_tensor(out=u_h[:], in0=delta[:],
                                            in1=xts[h][:], op=OP.mult)
                    dbx = big1pool.tile([128, N_, T_], DT_LIN, tag=f"dbx{h}")
                    nc.vector.tensor_tensor(
                        out=dbx[:],
                        in0=u_h[:].unsqueeze(1).broadcast_to([128, N_, T_]),
                        in1=bbc[:], op=OP.mult)
                    hbig = bigpool.tile([128, N_, T_], DT_LIN, tag=f"h{h}")
                    for n in range(N_):
                        idx = h * N_ + n
                        da = dapool.tile([128, T_], F32, tag="da")
                        nc.scalar.activation(da[:], delta[:], AF.Exp,
                                             scale=asc[:, idx:idx + 1])
                        init = 0.0 if k == 0 else state[:, idx:idx + 1]
                        nc.vector.tensor_tensor_scan(
                            out=hbig[:, n, :], data0=da[:], data1=dbx[:, n, :],
                            initial=init, op0=OP.mult, op1=OP.add)
                        nc.scalar.copy(state[:, idx:idx + 1],
                                       hbig[:, n, T_ - 1:T_])
                    yp = big1pool.tile([128, N_, T_], DT_LIN, tag=f"dbx{h}")
                    nc.vector.tensor_tensor(out=yp[:], in0=hbig[:], in1=cbc[:],
                                            op=OP.mult)
                    if REDUCE_ENG == "pool":
                        # n-contraction as a pairwise tree on GPSIMD (DVE is
                        # the critical path; Pool is otherwise idle).
                        q = bigpool.tile([128, N_ // 2, T_], F32, tag=f"h{h}")
                        nc.gpsimd.tensor_tensor(out=q[:], in0=yp[:, 0:N_:2, :],
                                                in1=yp[:, 1:N_:2, :], op=OP.add)
                        m = N_ // 2
                        while m > 1:
                            nc.gpsimd.tensor_tensor(
                                out=q[:, 0:m // 2, :], in0=q[:, 0:m // 2, :],
                                in1=q[:, m // 2:m, :], op=OP.add)
                            m //= 2
                        yv = q[:, 0, :]
                    else:
                        yvt = wpool.tile([128, T_], F32, tag=f"yv{h}")
                        nc.vector.tensor_reduce(out=yvt[:],
                                                in_=yp[:].transpose([0, 2, 1]),
                                                axis=mybir.AxisListType.X,
                                                op=OP.add)
                        yv = yvt[:]
                    # the x*D skip term is added on the host
                    nc.sync.dma_start(yt_d[h * 128:(h + 1) * 128, sl], yv)

    # All ACT funcs used (Exp, Ln, Copy) live in one table; restrict the
    # table list so insert_act_table_loads emits a single load instead of
    # ping-ponging between the exp-only and ln-only tables (1.3us each).
    import concourse.hw_specs as hw_specs
    orig_tables = hw_specs.get_activation_tables
    def _one_table(arch):
        # Keep every table at its original index (the emitted act_func_set_id
        # is positional), but strip Exp/Ln/Copy from all tables except
        # natural_log_exp_and_others so the chooser settles on that one.
        tabs = orig_tables(arch)
        keep = {AF.Exp, AF.Ln, AF.Copy}
        out = {}
        for name, funcs in tabs.items():
            if name == "natural_log_exp_and_others":
                out[name] = funcs
            else:
                out[name] = {f for f in funcs if f not in keep}
        return out
    hw_specs.get_activation_tables = _one_table
    try:
        import concourse.bacc as _bacc_mod
        _bacc_saved = _bacc_mod.get_activation_tables
        _bacc_mod.get_activation_tables = _one_table
        nc.compile()
    finally:
        hw_specs.get_activation_tables = orig_tables
        _bacc_mod.get_activation_tables = _bacc_saved
    _prog_cache["nc"] = nc
    return nc


def _permute_inputs(features, b, d):
    """Return the direction-d scan sequence of batch b as an (L, C) array."""
    f = np.asarray(features[b])                      # (C, H, W)
    if d == 0:
        return f.reshape(C_, L_).T
    if d == 1:
        return f.reshape(C_, L_).T[::-1]
    if d == 2:
        return f.transpose(1, 2, 0)[::-1].reshape(L_, C_)
    return f.transpose(2, 1, 0)[::-1].reshape(L_, C_)


def _unpermute_output(y, d):
    """Map a direction-d scan output (L, C) back to row-major (L, C)."""
    if d == 0:
        return y
    if d == 1:
        return y[::-1]
    if d == 2:
        return y.reshape(H_, W_, C_)[::-1].reshape(L_, C_)
    return y.reshape(W_, H_, C_)[::-1].transpose(1, 0, 2).reshape(L_, C_)


def kernel(features, A_log, D, W_delta, b_delta, W_B, W_C):
    features = np.asarray(features, np.float32)
    A_log = np.asarray(A_log, np.float32)
    D = np.asarray(D, np.float32)
    W_delta = np.asarray(W_delta, np.float32)
    b_delta = np.asarray(b_delta, np.float32)
    W_B = np.asarray(W_B, np.float32)
    W_C = np.asarray(W_C, np.float32)

    A = -np.exp(A_log)                                        # (C, N)
    asc = np.ascontiguousarray(np.concatenate([A[:128], A[128:]], axis=1))
    bsc = np.ascontiguousarray(b_delta.reshape(2, 128).T)

    in_maps = []
    for core in range(NCORES):
        b, d = divmod(core, 4)
        x = _permute_inputs(features, b, d)                   # (L, C)
        xt = np.ascontiguousarray(x.T, np.float32)
        brow = np.ascontiguousarray((x @ W_B).T).astype(ml_dtypes.bfloat16)
        crow = np.ascontiguousarray((x @ W_C).T).astype(ml_dtypes.bfloat16)
        in_maps.append({
            "xt": xt, "wd": W_delta,
            "brow": brow, "crow": crow, "asc": asc, "bsc": bsc,
        })

    nc = _build_program()
    res = run_bass_kernel_spmd(nc, in_maps, list(range(NCORES)))

    out = np.zeros((B_, L_, C_), np.float32)
    for core in range(NCORES):
        b, d = divmod(core, 4)
        x = _permute_inputs(features, b, d)
        y = res.results[core]["yt"].T + x * D                 # (L, C)
        out[b] += _unpermute_output(y, d)
    out /= 4.0
    return np.ascontiguousarray(out.transpose(0, 2, 1).reshape(B_, C_, H_, W_))


# revision 30
# speedup vs baseline: 1.0012x; 1.0012x over previous
"""Trainium2 Bass kernel for a 4-direction cross selective scan (VMamba SS2D).

Strategy: 8 NeuronCores, one (batch, direction) pair per core — B=2 x 4
directions. Each core runs an identical S6 selective-scan program over its
own pre-permuted (L, C) sequence; the host un-permutes and averages the four
directional outputs per batch.

Device layout: channels on partitions (two halves of 128), time on the free
dimension in chunks of T. The recurrence h_t = dA_t*h + dBx_t runs on the
native DVE tensor_tensor_scan (fp32 internal state), dA = exp(A*delta) on the
ACT engine with per-partition scale, softplus via exp+ln (one ACT table), and
the n-contraction y = sum_n C_n*h_n as a strided tensor_reduce.
"""

import sys

try:
    import concourse.bass as bass  # noqa: F401
except ImportError:
    sys.path.insert(0, "/opt/trn_rl_repo")

import numpy as np
import ml_dtypes
import concourse.mybir as mybir
import concourse.bacc as bacc
from concourse import tile
from concourse.bass_utils import run_bass_kernel_spmd

import os

B_, C_, H_, W_, N_ = 2, 256, 64, 64, 16
L_ = H_ * W_
T_ = int(os.environ.get("SS_T", 512))    # time-chunk width (free dim)
NCHUNK = L_ // T_
NCORES = 8
F32 = mybir.dt.float32
BF16 = mybir.dt.bfloat16
DT_LIN = BF16             # dtype of the linear factors (u, B, C, dBx, h, y')
AF = mybir.ActivationFunctionType
OP = mybir.AluOpType

_prog_cache = {}

NCHUNK_BUILD = int(os.environ.get("SS_NCHUNK", NCHUNK))
BC_VIA_DMA = os.environ.get("SS_BC", "dma") == "dma"
INPLACE_LN = os.environ.get("SS_LN", "sep") == "inplace"
REDUCE_ENG = os.environ.get("SS_RED", "pool")


def _build_program():
    if "nc" in _prog_cache:
        return _prog_cache["nc"]
    nc = bacc.Bacc("TRN2", target_bir_lowering=False, debug=False,
                   num_devices=NCORES)
    xt_d = nc.dram_tensor("xt", [C_, L_], F32, kind="ExternalInput")
    wd_d = nc.dram_tensor("wd", [C_, C_], F32, kind="ExternalInput")
    brow_d = nc.dram_tensor("brow", [N_, L_], DT_LIN, kind="ExternalInput")
    crow_d = nc.dram_tensor("crow", [N_, L_], DT_LIN, kind="ExternalInput")
    asc_d = nc.dram_tensor("asc", [128, 2 * N_], F32, kind="ExternalInput")
    bsc_d = nc.dram_tensor("bsc", [128, 2], F32, kind="ExternalInput")
    yt_d = nc.dram_tensor("yt", [C_, L_], F32, kind="ExternalOutput")

    with tile.TileContext(nc) as tc:
        with (
            tc.tile_pool(name="const", bufs=1) as cpool,
            tc.tile_pool(name="work", bufs=2) as wpool,
            tc.tile_pool(name="big", bufs=2) as bigpool,
            tc.tile_pool(name="big1", bufs=1) as big1pool,
            tc.tile_pool(name="da", bufs=4) as dapool,
            tc.tile_pool(name="one", bufs=1) as onepool,
            tc.tile_pool(name="psum", bufs=3, space="PSUM") as pspool,
        ):
            wd0 = cpool.tile([128, C_], F32, tag="wd0")
            nc.sync.dma_start(wd0[:], wd_d[0:128, :])
            wd1 = cpool.tile([128, C_], F32, tag="wd1")
            nc.sync.dma_start(wd1[:], wd_d[128:256, :])
            asc = cpool.tile([128, 2 * N_], F32, tag="asc")
            nc.sync.dma_start(asc[:], asc_d[:])
            bsc = cpool.tile([128, 2], F32, tag="bsc")
            nc.sync.dma_start(bsc[:], bsc_d[:])
            state = cpool.tile([128, 2 * N_], F32, tag="state")

            for k in range(NCHUNK_BUILD):
                sl = slice(k * T_, (k + 1) * T_)
                xts = []
                for h in range(2):
                    xt_h = wpool.tile([128, T_], F32, tag=f"xt{h}")
                    nc.sync.dma_start(xt_h[:], xt_d[h * 128:(h + 1) * 128, sl])
                    xts.append(xt_h)
                # Broadcast B/C time-rows to all 128 partitions straight from
                # DRAM (step-0 partition AP on the DMA source).
                bbc = bigpool.tile([128, N_, T_], DT_LIN, tag="bbc")
                cbc = bigpool.tile([128, N_, T_], DT_LIN, tag="cbc")
                if BC_VIA_DMA:
                    if k == 0:
                        # split the first broadcast so chunk 0's dBx (and the
                        # first scans) can start after half the transfer
                        nc.sync.dma_start(
                            bbc[:, 0:N_ // 2, :],
                            brow_d[0:N_ // 2, sl].unsqueeze(0)
                            .broadcast_to([128, N_ // 2, T_]))
                        nc.sync.dma_start(
                            bbc[:, N_ // 2:, :],
                            brow_d[N_ // 2:, sl].unsqueeze(0)
                            .broadcast_to([128, N_ // 2, T_]))
                    else:
                        nc.sync.dma_start(
                            bbc[:], brow_d[:, sl].unsqueeze(0)
                            .broadcast_to([128, N_, T_]))
                    nc.sync.dma_start(
                        cbc[:], crow_d[:, sl].unsqueeze(0).broadcast_to([128, N_, T_])
                    )
                else:
                    brr = wpool.tile([1, N_, T_], DT_LIN, tag="brr")
                    nc.sync.dma_start(brr[0:1, :, :], brow_d[:, sl])
                    nc.gpsimd.partition_broadcast(
                        bbc[:].rearrange("p n t -> p (n t)"),
                        brr[0:1].rearrange("p n t -> p (n t)"))
                    crr = wpool.tile([1, N_, T_], DT_LIN, tag="crr")
                    nc.sync.dma_start(crr[0:1, :, :], crow_d[:, sl])
                    nc.gpsimd.partition_broadcast(
                        cbc[:].rearrange("p n t -> p (n t)"),
                        crr[0:1].rearrange("p n t -> p (n t)"))
                for h in range(2):
                    psd = pspool.tile([128, T_], F32, tag="psd")
                    nc.tensor.matmul(psd[:], wd0[:, h * 128:(h + 1) * 128],
                                     xts[0][:], start=True, stop=False)
                    nc.tensor.matmul(psd[:], wd1[:, h * 128:(h + 1) * 128],
                                     xts[1][:], start=False, stop=True)
                    # softplus(z + b) = ln(1 + exp(z + b)); Exp and Ln live in
                    # the same ACT table so no table reloads.
                    delta = wpool.tile([128, T_], F32, tag=f"delta{h}")
                    if INPLACE_LN:
                        nc.scalar.activation(delta[:], psd[:], AF.Exp,
                                             bias=bsc[:, h:h + 1])
                        nc.scalar.activation(delta[:], delta[:], AF.Ln, bias=1.0)
                    else:
                        esb = onepool.tile([128, T_], F32, tag="esb")
                        nc.scalar.activation(esb[:], psd[:], AF.Exp,
                                             bias=bsc[:, h:h + 1])
                        nc.scalar.activation(delta[:], esb[:], AF.Ln, bias=1.0)
                    u_h = wpool.tile([128, T_], DT_LIN, tag=f"u{h}")
                    nc.gpsimd.tensor_tensor(out=u_h[:], in0=delta[:],
                                            in1=xts[h][:], op=OP.mult)
                    dbx = big1pool.tile([128, N_, T_], DT_LIN, tag=f"dbx{h}")
                    if k == 0:
                        for half in range(2):
                            ns = slice(half * N_ // 2, (half + 1) * N_ // 2)
                            nc.vector.tensor_tensor(
                                out=dbx[:, ns, :],
                                in0=u_h[:].unsqueeze(1)
                                .broadcast_to([128, N_ // 2, T_]),
                                in1=bbc[:, ns, :], op=OP.mult)
                    else:
                        nc.vector.tensor_tensor(
                            out=dbx[:],
                            in0=u_h[:].unsqueeze(1).broadcast_to([128, N_, T_]),
                            in1=bbc[:], op=OP.mult)
                    hbig = bigpool.tile([128, N_, T_], DT_LIN, tag=f"h{h}")
                    for n in range(N_):
                        idx = h * N_ + n
                        da = dapool.tile([128, T_], F32, tag="da")
                        nc.scalar.activation(da[:], delta[:], AF.Exp,
                                             scale=asc[:, idx:idx + 1])
                        init = 0.0 if k == 0 else state[:, idx:idx + 1]
                        nc.vector.tensor_tensor_scan(
                            out=hbig[:, n, :], data0=da[:], data1=dbx[:, n, :],
                            initial=init, op0=OP.mult, op1=OP.add)
                        nc.scalar.copy(state[:, idx:idx + 1],
                                       hbig[:, n, T_ - 1:T_])
                    yp = big1pool.tile([128, N_, T_], DT_LIN, tag=f"dbx{h}")
                    nc.vector.tensor_tensor(out=yp[:], in0=hbig[:], in1=cbc[:],
                                            op=OP.mult)
                    if REDUCE_ENG == "pool":
                        # n-contraction as a pairwise tree on GPSIMD (DVE is
                        # the critical path; Pool is otherwise idle).
                        q = bigpool.tile([128, N_ // 2, T_], F32, tag=f"h{h}")
                        nc.gpsimd.tensor_tensor(out=q[:], in0=yp[:, 0:N_:2, :],
                                                in1=yp[:, 1:N_:2, :], op=OP.add)
                        m = N_ // 2
                        while m > 1:
                            nc.gpsimd.tensor_tensor(
                                out=q[:, 0:m // 2, :], in0=q[:, 0:m // 2, :],
                                in1=q[:, m // 2:m, :], op=OP.add)
                            m //= 2
                        yv = q[:, 0, :]
                    else:
                        yvt = wpool.tile([128, T_], F32, tag=f"yv{h}")
                        nc.vector.tensor_reduce(out=yvt[:],
                                                in_=yp[:].transpose([0, 2, 1]),
                                                axis=mybir.AxisListType.X,
                                                op=OP.add)
                        yv = yvt[:]
                    # the x*D skip term is added on the host
                    nc.sync.dma_start(yt_d[h * 128:(h + 1) * 128, sl], yv)

    # All ACT funcs used (Exp, Ln, Copy) live in one table; restrict the
    # table list so insert_act_table_loads emits a single load instead of
    # ping-ponging between the exp-only and ln-only tables (1.3us each).
    import concourse.hw_specs as hw_specs
    orig_tables = hw_specs.get_activation_tables
    def _one_table(arch):
        # Keep every table at its original index (the emitted act_func_set_id
        # is positional), but strip Exp/Ln/Copy from all tables except
        # natural_log_exp_and_others so the chooser settles on that one.
        tabs = orig_tables(arch)
        keep = {AF.Exp, AF.Ln, AF.Copy}
        out = {}
        for name, funcs in tabs.items():
            if name == "natural_log_exp_and_others":
                out[name] = funcs
            else:
                out[name] = {f for f in funcs if f not in keep}
        return out
    hw_specs.get_activation_tables = _one_table
    try:
        import concourse.bacc as _bacc_mod
        _bacc_saved = _bacc_mod.get_activation_tables
        _bacc_mod.get_activation_tables = _one_table
        nc.compile()
    finally:
        hw_specs.get_activation_tables = orig_tables
        _bacc_mod.get_activation_tables = _bacc_saved
    _prog_cache["nc"] = nc
    return nc


def _permute_inputs(features, b, d):
    """Return the direction-d scan sequence of batch b as an (L, C) array."""
    f = np.asarray(features[b])                      # (C, H, W)
    if d == 0:
        return f.reshape(C_, L_).T
    if d == 1:
        return f.reshape(C_, L_).T[::-1]
    if d == 2:
        return f.transpose(1, 2, 0)[::-1].reshape(L_, C_)
    return f.transpose(2, 1, 0)[::-1].reshape(L_, C_)


def _unpermute_output(y, d):
    """Map a direction-d scan output (L, C) back to row-major (L, C)."""
    if d == 0:
        return y
    if d == 1:
        return y[::-1]
    if d == 2:
        return y.reshape(H_, W_, C_)[::-1].reshape(L_, C_)
    return y.reshape(W_, H_, C_)[::-1].transpose(1, 0, 2).reshape(L_, C_)


def kernel(features, A_log, D, W_delta, b_delta, W_B, W_C):
    features = np.asarray(features, np.float32)
    A_log = np.asarray(A_log, np.float32)
    D = np.asarray(D, np.float32)
    W_delta = np.asarray(W_delta, np.float32)
    b_delta = np.asarray(b_delta, np.float32)
    W_B = np.asarray(W_B, np.float32)
    W_C = np.asarray(W_C, np.float32)

    A = -np.exp(A_log)                                        # (C, N)
    asc = np.ascontiguousarray(np.concatenate([A[:128], A[128:]], axis=1))
    bsc = np.ascontiguousarray(b_delta.reshape(2, 128).T)

    in_maps = []
    for core in range(NCORES):
        b, d = divmod(core, 4)
        x = _permute_inputs(features, b, d)                   # (L, C)
        xt = np.ascontiguousarray(x.T, np.float32)
        brow = np.ascontiguousarray((x @ W_B).T).astype(ml_dtypes.bfloat16)
        crow = np.ascontiguousarray((x @ W_C).T).astype(ml_dtypes.bfloat16)
        in_maps.append({
            "xt": xt, "wd": W_delta,
            "brow": brow, "crow": crow, "asc": asc, "bsc": bsc,
        })

    nc = _build_program()
    res = run_bass_kernel_spmd(nc, in_maps, list(range(NCORES)))

    out = np.zeros((B_, L_, C_), np.float32)
    for core in range(NCORES):
        b, d = divmod(core, 4)
        x = _permute_inputs(features, b, d)
        y = res.results[core]["yt"].T + x * D                 # (L, C)
        out[b] += _unpermute_output(y, d)
    out /= 4.0
    return np.ascontiguousarray(out.transpose(0, 2, 1).reshape(B_, C_, H_, W_))
